# revision 38
# baseline (speedup 1.0000x reference)
"""Trainium2 Bass kernel for nn_DocREModel (8-core SPMD).

Sharding: data-parallel over the 4 documents x 2 pair-halves = 8 cores.
Each core runs an identical program; per-core behavior differs only via
its input data (its doc's tensors + its half of the pair one-hots).

All floating-point arithmetic runs on device. Host does only index-driven
data movement: batch slicing, transposes, row gathers at integer indices,
and one-hot/selector matrix construction.

DMA plan: all inputs stream on the sync-engine HWDGE queue in strict
consumption order (xt/wtrans interleaved per k-pair, adjacency, xrow,
attm, selectors, wrel, then the conv/classifier weight streams); the
output leaves on the scalar-engine queue so it never drains behind the
bulk stream. Bias vectors ride as rows of one packed tensor and are
transposed on-device right before the conv stack (off the S1 critical
path). The RGCN computes only the 22 entity rows consumed downstream,
with adjacency slimmed to the matching 110 columns.
"""

import numpy as np
from contextlib import ExitStack

import concourse.bass as bass
import concourse.bacc as bacc
import concourse.tile as tile
import concourse.mybir as mybir
from concourse.bass_utils import run_bass_kernel_spmd

FP32 = mybir.dt.float32
BF16 = mybir.dt.bfloat16
FP8 = mybir.dt.float8e4
DR = mybir.MatmulPerfMode.DoubleRow

SEQ_DT = BF16    # seq_t tiles (hold 64*seq)
CONV_DT = BF16   # conv stack
PAIR_DT = BF16   # pair-classification matmuls
GRAPH_DT = BF16  # rgcn matmuls
WS = 64.0        # W_trans fp8 scale; seq psums come out 64x

import ml_dtypes

_NPDT = {FP32: np.float32, BF16: ml_dtypes.bfloat16, FP8: ml_dtypes.float8_e4m3}

B, C, H, NH = 4, 1024, 768, 12
E, M, L, LS = 22, 3, 30, 16
NN, NF, EMB = 118, 532, 512
P = 462
CAP = 256         # per-core pair capacity (row-routed halves, padded)
IC = 256
S = 22            # spatial side of relation map
SP = S * S        # 484
PW = 32           # padded side (32 for aligned rows)
SPP = PW * PW     # 676
# spatial halo split: this core computes conv3 rows 0..10 of its (possibly
# flipped) relation map; conv2 rows 0..12, conv1 rows 0..14 feed it.
R1, R2, R3 = 15, 13, 11
C1, C2, C3 = R1 * S, R2 * S, R3 * S   # 330, 286, 242
ACT = mybir.ActivationFunctionType
KT_H = H // 128   # 6
ATTM_ROWS = E * M * NH          # 792
SPAN_ROWS = L * LS              # 480
SPAN_TILES = [128, 128, 128, 96]
NF_TILES = [128, 128, 128, 128, 20]   # 532
SP3_TILES = [128, C3 - 128]           # 242
AC = 110          # slim adjacency cols: 4 relations x 22 entities + 22 self
CA = 1056         # attm packed half-width: 1024 att cols + 32 gmat cols
XR = 990          # xrow packed width: 768 xspan + 192 attl + 30 gspan
NBIAS = 17        # biasT rows: b1(2) b2(2) b3(4) bht(8) bbil(1)


def _ts(sizes):
    """(offset, size) pairs for a tiling."""
    off = 0
    for sz in sizes:
        yield off, sz
        off += sz


def build_program():
    nc = bacc.Bacc("TRN2", target_bir_lowering=False, debug=False)

    dins = {}

    def din(name, shape, dt=FP32):
        dins[name] = nc.dram_tensor(name, shape, dt, kind="ExternalInput").ap()
        return dins[name]

    CX = C + 128    # xt carries the 66 mention columns + zero pad to 128
    # --- bulk streams, packed into long rows for DMA packet efficiency ---
    xt = din("xt", [H // 2, 2 * CX], FP8)     # [X.T | X[mention].T | 0]
    xrow = din("xrow", [SPAN_ROWS, XR], BF16)  # [xspan | attl | gspan]
    attm = din("attm", [512, 2 * CA], FP8)    # mention att rows + gmat cols
    w1t = din("w1t", [EMB, 25 * IC], CONV_DT)
    w3t = din("w3t", [IC, 25 * EMB], CONV_DT)
    wht = din("wht", [4 * EMB, 2 * EMB], PAIR_DT)
    wtrans = din("wtrans", [H // 2, 2 * EMB], FP8)  # 64x scaled
    btrans = din("btrans", [1, EMB], FP32)    # 64x scaled
    ident = din("ident", [128, 128], FP32)
    identp = din("identp", [128, 128], PAIR_DT)
    biasT = din("biasT", [NBIAS, 128], FP32)  # bias vectors as rows
    brgcn = din("brgcn", [1, EMB], FP32)
    typ = din("typ", [NN, 20], GRAPH_DT)      # type_embed[node_types]
    g3 = din("g3", [E * M, E], GRAPH_DT)      # kron(I22, ones(3))
    adjt = din("adjt", [NN, AC], FP32)        # [j, (r, i<22)] + self cols
    wrel = din("wrel", [NF, 5 * EMB], GRAPH_DT)   # rows k, cols (r, emb)
    w2t = din("w2t", [IC, 25 * IC], CONV_DT)
    sh = din("sh", [E, CAP], PAIR_DT)
    st = din("st", [E, CAP], PAIR_DT)
    sm = din("sm", [C3, CAP], PAIR_DT)
    wbil = din("wbil", [128, 8 * 97], PAIR_DT)    # rows k%128, cols (kt, 97)
    outt = nc.dram_tensor("outt", [97, CAP], FP32, kind="ExternalOutput").ap()

    with tile.TileContext(nc) as tc, ExitStack() as ctx:
        pp = ctx.enter_context(tc.tile_pool(name="persist", bufs=1))
        pst = ctx.enter_context(tc.tile_pool(name="stream", bufs=1))
        pps = ctx.enter_context(tc.tile_pool(name="psum", bufs=8, space="PSUM"))
        pdram = ctx.enter_context(tc.tile_pool(name="dram", bufs=1, space="DRAM"))

        dma = nc.sync.dma_start      # bulk queue (consumption order)
        dmas = nc.scalar.dma_start   # output only
        dmag = nc.gpsimd.dma_start   # warm-up flush only

        def T(pool, shape, dt, tag, bufs=None):
            return pool.tile(shape, dt, tag=tag, name=tag, bufs=bufs)

        # ====== sync-queue DMAs, strict consumption order ======
        # S1 feeds first, interleaved per k-pair so compute pipelines
        wtrans_t, xt_t = [], []
        for kp in range(KT_H // 2):
            t = T(pp, [128, 2 * CX], FP8, f"xt{kp}")
            dma(t[:], xt[kp * 128:(kp + 1) * 128, :])
            xt_t.append(t[:])
            t = T(pp, [128, 2 * EMB], FP8, f"wtrans{kp}")
            dma(t[:], wtrans[kp * 128:(kp + 1) * 128, :])
            wtrans_t.append(t[:])
        ident_t = T(pp, [128, 128], FP32, "ident")
        dma(ident_t[:], ident)
        identp_t = T(pp, [128, 128], PAIR_DT, "identp")
        dma(identp_t[:], identp)
        # adjacency (slim) + small persistent tensors
        adjt_t = T(pp, [NN, AC], FP32, "adjt")
        dma(adjt_t[:], adjt)
        NODE_GROUPS = [(0, E), (E, E * M), (E + E * M, L)]
        adjf_t = []
        for gi, (goff, gsz) in enumerate(NODE_GROUPS):
            tf = T(pp, [gsz, AC], FP32, f"adjf{gi}")
            dma(tf[:], adjt[goff:goff + gsz, :])
            adjf_t.append(tf)
        biasT_t = T(pp, [NBIAS, 128], FP32, "biasT")
        dma(biasT_t[:], biasT)
        btrans_t = T(pp, [1, EMB], FP32, "btrans")
        dma(btrans_t[:], btrans)
        brgcn_t = T(pp, [1, EMB], FP32, "brgcn")
        dma(brgcn_t[:], brgcn)
        nodes_e = T(pp, [E, NF], GRAPH_DT, "nodes_e")
        nodes_m = T(pp, [E * M, NF], GRAPH_DT, "nodes_m")
        nodes_l = T(pp, [L, NF], GRAPH_DT, "nodes_l")
        dma(nodes_e[:, EMB:NF], typ[0:E, :])
        dma(nodes_m[:, EMB:NF], typ[E:E + E * M, :])
        dma(nodes_l[:, EMB:NF], typ[E + E * M:NN, :])
        g3_t = T(pp, [E * M, E], GRAPH_DT, "g3")
        dma(g3_t[:], g3)
        node_tiles = [nodes_e, nodes_m, nodes_l]
        xrow_t = []
        for i, (off, sz) in enumerate(_ts(SPAN_TILES)):
            t = T(pp, [sz, XR], BF16, f"xrow{i}")
            dma(t[:], xrow[off:off + sz, :])
            xrow_t.append(t[:])
        attm_t = []
        for kp in range(4):
            t = T(pp, [128, 2 * CA], FP8, f"attm{kp}")
            dma(t[:], attm[kp * 128:(kp + 1) * 128, :])
            attm_t.append(t[:])
        # pair selectors + rgcn weights
        sh_t = T(pp, [E, CAP], PAIR_DT, "sh")
        dma(sh_t[:], sh)
        st_t = T(pp, [E, CAP], PAIR_DT, "st")
        dma(st_t[:], st)
        sm_t = []
        for i, (off, sz) in enumerate(_ts(SP3_TILES)):
            t = T(pp, [sz, CAP], PAIR_DT, f"sm{i}")
            dma(t[:], sm[off:off + sz, :])
            sm_t.append(t)
        wrel_t = []
        for i, (off, sz) in enumerate(_ts(NF_TILES)):
            t = T(pp, [sz, 5 * EMB], GRAPH_DT, f"wrel{i}")
            dma(t[:], wrel[off:off + sz, :])
            wrel_t.append(t)
        wbil_t = T(pp, [128, 8 * 97], PAIR_DT, "wbil")
        dma(wbil_t[:], wbil)

        # ================= PE warm-up (no DMA dependency) ================
        warm_in = T(pp, [128, 128], PAIR_DT, "warm_in")
        nc.vector.memset(warm_in[:], 0.25)
        ps_warm = T(pps, [128, 128], FP32, "ps")
        for _ in range(52):
            nc.tensor.matmul(ps_warm[:], warm_in[:], warm_in[:],
                             start=True, stop=True)

        ones_t = T(pp, [128, 1], FP32, "ones")
        nc.vector.memset(ones_t[:], 1.0)
        btrans_bc = T(pp, [128, EMB], FP32, "btrans_bc")
        nc.gpsimd.partition_broadcast(btrans_bc[:], btrans_t[:])
        brgcn_bc = T(pp, [E, EMB], FP32, "brgcn_bc")
        nc.gpsimd.partition_broadcast(brgcn_bc[:], brgcn_t[:])
        # ---- S1: seq = X @ W_trans + b  (fp8 DR; seq_t tiles hold 64*seq) --
        KP_H = KT_H // 2  # 3 k-pairs over H
        ps_seq = [T(pps, [128, EMB], FP32, "ps") for _ in range(8)]
        xgp_t = []
        for kp in range(KP_H):
            xtv = xt_t[kp].rearrange("p (two c) -> p two c", two=2)
            wtv = wtrans_t[kp].rearrange("p (two c) -> p two c", two=2)
            for mt in range(8):
                nc.tensor.matmul(
                    ps_seq[mt][:], xtv[:, :, mt * 128:(mt + 1) * 128], wtv,
                    start=(kp == 0), stop=(kp == KP_H - 1), perf_mode=DR)
            xg_k = T(pp, [128, 2 * 128], FP8, f"xgp{kp}")
            nc.scalar.copy(
                xg_k[:].rearrange("p (two c) -> p two c", two=2),
                xtv[:, :, C:CX])
            xgp_t.append(xg_k)
        seq_t = []
        for mt in range(8):
            t = T(pp, [128, EMB], SEQ_DT, f"seq{mt}")
            nc.vector.tensor_copy(t[:], ps_seq[mt][:])
            seq_t.append(t)

        # ---- S2: mention embeddings + entity logsumexp nodes ----
        ps_memb = T(pps, [128, EMB], FP32, "ps")
        for kp in range(KP_H):
            nc.tensor.matmul(
                ps_memb[:],
                xgp_t[kp][:].rearrange("p (two c) -> p two c", two=2),
                wtrans_t[kp].rearrange("p (two c) -> p two c", two=2),
                start=(kp == 0), stop=(kp == KP_H - 1), perf_mode=DR)
        memb_t = T(pp, [E * M, EMB], FP32, "memb")
        nc.vector.tensor_add(memb_t[:], ps_memb[0:E * M, :],
                             btrans_bc[0:E * M, :])
        nc.vector.tensor_scalar_mul(memb_t[:], memb_t[:], 1.0 / WS)
        nc.vector.tensor_copy(nodes_m[:, 0:EMB], memb_t[:])
        ememb_t = T(pp, [E * M, EMB], GRAPH_DT, "ememb")
        nc.scalar.activation(ememb_t[:], memb_t[:], ACT.Exp)
        ps_ent = T(pps, [E, EMB], FP32, "ps")
        nc.tensor.matmul(ps_ent[:], g3_t[:], ememb_t[:], start=True, stop=True)
        nc.scalar.activation(nodes_e[:, 0:EMB], ps_ent[:], ACT.Ln)

        # ---- S3: link nodes (packed xrow tiles: xspan | attl | gspan) ----
        aT_t, aTb_t = [], []
        for i, (off, sz) in enumerate(_ts(SPAN_TILES)):
            xr = xrow_t[i]
            a = T(pp, [sz, 1], FP32, f"aT{i}")
            nc.vector.tensor_reduce(a[:], xr[:, 768:960], mybir.AxisListType.X,
                                    mybir.AluOpType.add)
            nc.vector.tensor_scalar_mul(a[:], a[:], 1.0 / (NH * LS))
            aT_t.append(a)
            ab = T(pp, [sz, 1], SEQ_DT, f"aTb{i}")
            nc.vector.tensor_copy(ab[:], a[:])
            aTb_t.append(ab)
        # asum[l] = sum_j a_l[j] (for the bias term); uses unscaled-by-X a
        ps_as = T(pps, [L, 1], FP32, "ps")
        for kt in range(4):
            nc.tensor.matmul(ps_as[:], xrow_t[kt][:, 960:990], aTb_t[kt][:],
                             start=(kt == 0), stop=(kt == 3))
        asum_t = T(pp, [L, 1], FP32, "asum")
        nc.vector.tensor_copy(asum_t[:], ps_as[:])
        # scale xspan rows by a in place, then project through gspan
        for kt in range(4):
            nc.vector.tensor_scalar_mul(xrow_t[kt][:, 0:768],
                                        xrow_t[kt][:, 0:768], aT_t[kt][:])
        # linkctxT [768, 30] as fp8 k-pairs (L padded to 32)
        LP = 32
        lct_t = []
        for kp in range(KP_H):
            t = T(pp, [128, 2 * LP], FP8, f"lct{kp}")
            nc.vector.memset(t[:], 0.0)
            lct_t.append(t)
        for mt in range(KT_H):
            ps = T(pps, [128, L], FP32, "ps")
            for kt in range(4):
                nc.tensor.matmul(ps[:],
                                 xrow_t[kt][:, mt * 128:(mt + 1) * 128],
                                 xrow_t[kt][:, 960:990],
                                 start=(kt == 0), stop=(kt == 3))
            kp, ih = divmod(mt, 2)
            nc.vector.tensor_copy(lct_t[kp][:, ih * LP:ih * LP + L], ps[:])
        bterm_t = T(pp, [L, EMB], FP32, "bterm")
        nc.vector.tensor_scalar_mul(bterm_t[:], btrans_bc[0:L, :], asum_t[:])
        ps_link = T(pps, [LP, EMB], FP32, "ps")
        for kp in range(KP_H):
            nc.tensor.matmul(
                ps_link[:],
                lct_t[kp][:].rearrange("p (two c) -> p two c", two=2),
                wtrans_t[kp].rearrange("p (two c) -> p two c", two=2),
                start=(kp == 0), stop=(kp == KP_H - 1), perf_mode=DR)
        nc.vector.tensor_add(nodes_l[:, 0:EMB], ps_link[0:L, :], bterm_t[:])
        nc.vector.tensor_scalar_mul(nodes_l[:, 0:EMB], nodes_l[:, 0:EMB],
                                    1.0 / WS)

        # ---- S4: ea (entity attention) + e_ctx ----
        # attm fp8 DR, rows padded 792 -> 1024 (4 k-pairs); gmat columns are
        # packed alongside (cols 1024:1056 of each half).
        ps_ea = [T(pps, [32, 512], FP32, "ps") for _ in range(2)]
        for kp in range(4):
            atv = attm_t[kp].rearrange("p (two c) -> p two c", two=2)
            gtv = atv[:, :, 1024:1056]
            for half in range(2):
                nc.tensor.matmul(ps_ea[half][:], gtv,
                                 atv[:, :, half * 512:(half + 1) * 512],
                                 start=(kp == 0), stop=(kp == 3), perf_mode=DR)
        ea_t = T(pp, [E, C], FP32, "ea")
        for half in range(2):
            nc.vector.tensor_copy(ea_t[:, half * 512:(half + 1) * 512],
                                  ps_ea[half][0:E, :])
        rsum_t = T(pp, [E, 1], FP32, "rsum")
        nc.vector.tensor_reduce(rsum_t[:], ea_t[:], mybir.AxisListType.X,
                                mybir.AluOpType.add)
        nc.vector.tensor_scalar_add(rsum_t[:], rsum_t[:], 1e-5 * NH * M)
        recip_t = T(pp, [E, 1], FP32, "recip")
        nc.vector.reciprocal(recip_t[:], rsum_t[:])
        # fold the 1/64 seq descale into the normalization
        nc.vector.tensor_scalar_mul(recip_t[:], recip_t[:], 1.0 / WS)
        ean_t = ea_t
        nc.vector.tensor_scalar_mul(ean_t[:], ea_t[:], recip_t[:])
        # eaNT via PE transpose (all up front), then e_ctx [22, 512]
        eaT_t = []
        for kt in range(8):
            pst_ea = T(pps, [128, E], FP32, "ps")
            nc.tensor.transpose(pst_ea[:], ean_t[:, kt * 128:(kt + 1) * 128],
                                ident_t[0:E, 0:E])
            eaT = T(pst, [128, E], SEQ_DT, "eaT_stream", bufs=8)
            nc.vector.tensor_copy(eaT[:], pst_ea[:])
            eaT_t.append(eaT)
        sfrac_t = T(pp, [E, 1], FP32, "sfrac")
        nc.vector.tensor_mul(sfrac_t[:], rsum_t[:], recip_t[:])
        bterm2_t = T(pp, [E, EMB], FP32, "bterm2")
        nc.vector.tensor_scalar_mul(bterm2_t[:], btrans_bc[0:E, :], sfrac_t[:])
        ps_ectx = [T(pps, [E, EMB], FP32, "ps") for _ in range(2)]
        for kt in range(8):
            nc.tensor.matmul(ps_ectx[kt % 2][:], eaT_t[kt][:], seq_t[kt][:],
                             start=(kt < 2), stop=(kt >= 6))
        ectx_a = T(pp, [E, EMB], FP32, "ectx_a")
        nc.vector.tensor_add(ectx_a[:], ps_ectx[0][:], bterm2_t[:])
        ectx_t = T(pp, [E, EMB], FP32, "ectx")
        nc.vector.tensor_add(ectx_t[:], ps_ectx[1][:], ectx_a[:])

        # ---- adjacency normalize (slim: only entity rows i<22 needed) ----
        ps_rs = T(pps, [1, AC], FP32, "ps")
        nc.tensor.matmul(ps_rs[:], ones_t[0:NN, 0:1], adjt_t[:, :],
                         start=True, stop=True)
        rs_t = T(pp, [1, AC], FP32, "rs")
        nc.vector.tensor_scalar_add(rs_t[:], ps_rs[:], 1e-5)
        rcp_t = T(pp, [1, AC], FP32, "rcp")
        nc.vector.reciprocal(rcp_t[:], rs_t[:])
        rsbc_t = T(pp, [128, AC], FP32, "rsbc")
        nc.gpsimd.partition_broadcast(rsbc_t[:], rcp_t[:])
        adjn_t = []
        for gi, (goff, gsz) in enumerate(NODE_GROUPS):
            t = T(pp, [gsz, AC], GRAPH_DT, f"adjn{gi}")
            nc.vector.tensor_mul(t[:], adjf_t[gi][:], rsbc_t[0:gsz, :])
            adjn_t.append(t)

        # ---- S5: RGCN (entity rows only) ----
        # msgT[k, (r, i<22)] for 4 relations + self, one psum per NF tile
        msg_t = []
        for i, (off, sz) in enumerate(_ts(NF_TILES)):
            ps = T(pps, [sz, AC], FP32, "ps")
            for gi, (goff, gsz) in enumerate(NODE_GROUPS):
                nc.tensor.matmul(ps[:], node_tiles[gi][:, off:off + sz],
                                 adjn_t[gi][:, :],
                                 start=(gi == 0), stop=(gi == 2))
            t = T(pp, [sz, AC], GRAPH_DT, f"msg{i}")
            nc.vector.tensor_copy(t[:], ps[:])
            msg_t.append(t)
        # gcn[e<22, :] = relu(sum_r msg_r^T @ Wrel_r + b)
        # two alternating psums break the serial accumulation chain
        ps_gcn = [T(pps, [E, EMB], FP32, "ps") for _ in range(2)]
        for n, (i, r) in enumerate(
                (i, r) for i in range(len(NF_TILES)) for r in range(5)):
            nc.tensor.matmul(
                ps_gcn[n % 2][:], msg_t[i][:, r * E:(r + 1) * E],
                wrel_t[i][:, r * EMB:(r + 1) * EMB],
                start=(n < 2), stop=(n >= 23))
        gcn_ps = T(pp, [E, EMB], FP32, "gcn_ps")
        nc.vector.tensor_add(gcn_ps[:], ps_gcn[0][:], brgcn_bc[:])
        gcn_sb = T(pp, [E, EMB], FP32, "gcn_sb")
        nc.vector.tensor_add(gcn_sb[:], ps_gcn[1][:], gcn_ps[:])
        gcn_r = T(pp, [E, EMB], FP32, "gcn_r")
        nc.scalar.activation(gcn_r[:], gcn_sb[:], ACT.Relu)
        ent_t = T(pp, [E, EMB], PAIR_DT, "ent")
        nc.vector.tensor_copy(ent_t[:], gcn_r[:])
        # entT tiles [128, 22] x4 (for the outer-product relation maps)
        gcnT_t = []
        for mt in range(4):
            ps = T(pps, [128, E], FP32, "ps")
            nc.tensor.transpose(ps[:], gcn_r[:, mt * 128:(mt + 1) * 128],
                                ident_t[0:E, 0:E])
            t = T(pp, [128, E], FP32, f"gcnT{mt}")
            nc.vector.tensor_copy(t[:], ps[:])
            gcnT_t.append(t)
        # ectxT tiles [128, 22] x4
        ectxT_t = []
        for mt in range(4):
            ps = T(pps, [128, E], FP32, "ps")
            nc.tensor.transpose(ps[:], ectx_t[:, mt * 128:(mt + 1) * 128],
                                ident_t[0:E, 0:E])
            t = T(pp, [128, E], FP32, f"ectxT{mt}")
            nc.vector.tensor_copy(t[:], ps[:])
            ectxT_t.append(t)

        # ---- pair features that only need ent: hs, ts, hs*ts ----
        featT = [None] * 16
        for mt in range(4):
            ps = T(pps, [128, CAP], FP32, "ps")
            nc.tensor.matmul(ps[:], ent_t[:, mt * 128:(mt + 1) * 128], sh_t[:],
                             start=True, stop=True)
            t = T(pp, [128, CAP], PAIR_DT, f"featT{mt}")
            nc.vector.tensor_copy(t[:], ps[:])
            featT[mt] = t
        for mt in range(4):
            ps = T(pps, [128, CAP], FP32, "ps")
            nc.tensor.matmul(ps[:], ent_t[:, mt * 128:(mt + 1) * 128], st_t[:],
                             start=True, stop=True)
            t = T(pp, [128, CAP], PAIR_DT, f"featT{4 + mt}")
            nc.vector.tensor_copy(t[:], ps[:])
            featT[4 + mt] = t
        for mt in range(4):
            t = T(pp, [128, CAP], PAIR_DT, f"featT{12 + mt}")
            nc.vector.tensor_mul(t[:], featT[mt][:], featT[4 + mt][:])
            featT[12 + mt] = t

        # ---- S6: relation map x + conv stack ----
        xpad_t = []
        for mt in range(4):
            xp = T(pp, [128, SPP], CONV_DT, f"xpad{mt}")
            nc.vector.memset(xp[:], 0.0)
            t1 = T(pp, [128, SP], CONV_DT, "xtmp1")
            nc.vector.tensor_mul(
                t1[:].rearrange("p (a b) -> p a b", a=S, b=S),
                gcnT_t[mt][:].unsqueeze(2).to_broadcast((128, S, S)),
                gcnT_t[mt][:].unsqueeze(1).to_broadcast((128, S, S)))
            t2 = T(pp, [128, SP], CONV_DT, "xtmp2")
            nc.vector.tensor_mul(
                t2[:].rearrange("p (a b) -> p a b", a=S, b=S),
                ectxT_t[mt][:].unsqueeze(2).to_broadcast((128, S, S)),
                ectxT_t[mt][:].unsqueeze(1).to_broadcast((128, S, S)))
            inner = xp[:].rearrange("p (a b) -> p a b", a=PW, b=PW)[:, 2:2 + S, 2:2 + S]
            nc.vector.tensor_add(inner, t1[:], t2[:])
            xpad_t.append(xp)

        # bias columns: transpose biasT rows -> bsb [128, NBIAS]
        ps_bias = T(pps, [128, NBIAS], FP32, "ps")
        nc.tensor.transpose(ps_bias[:], biasT_t[:], ident_t[0:NBIAS, 0:NBIAS])
        bsb = T(pp, [128, NBIAS], FP32, "bsb")
        nc.vector.tensor_copy(bsb[:], ps_bias[:])

        def conv(in_tiles, wsrc, n_ic_t, n_oc_t, ocs, rows, out_cb, tgsz,
                 nacc=1):
            """5x5 SAME conv producing rows 0..rows-1 of the 22-col map.

            wsrc(kt, tg) returns the [128, tgsz*ocs] weight slab AP.
            nacc>1 splits each oc tile's accumulation over multiple psum
            banks (tap parity) to decouple the serial accumulate chain."""
            ps_c = [[T(pps, [128, rows * S], FP32, "ps") for _ in range(nacc)]
                    for _ in range(n_oc_t)]
            n_acc = 25 * n_ic_t
            a = 0
            for kt in range(n_ic_t):
                for tg in range(25 // tgsz):
                    w = wsrc(kt, tg)
                    for tl in range(tgsz):
                        tap = tg * tgsz + tl
                        di, dj = divmod(tap, 5)
                        rhs = in_tiles[kt][:].rearrange(
                            "p (a b) -> p a b",
                            a=PW, b=PW)[:, di:di + rows, dj:dj + S]
                        for mt in range(n_oc_t):
                            nc.tensor.matmul(
                                ps_c[mt][a % nacc][:],
                                w[:, tl * ocs + mt * 128:tl * ocs + (mt + 1) * 128],
                                rhs, start=(a < nacc),
                                stop=(a >= n_acc - nacc))
                        a += 1
            for mt in range(n_oc_t):
                if nacc == 1:
                    out_cb(mt, ps_c[mt][0])
                else:
                    tmp = T(pp, [128, rows * S], FP32, "cmerge", bufs=1)
                    nc.vector.tensor_copy(tmp[:], ps_c[mt][0][:])
                    tmp2 = T(pp, [128, rows * S], FP32, "cmerge2", bufs=1)
                    nc.vector.tensor_add(tmp2[:], ps_c[mt][1][:], tmp[:])
                    out_cb(mt, tmp2)

        def stream_w(w_dram, ocs, tgsz, bufs, tag="wconv_stream"):
            def src(kt, tg):
                w = T(pst, [128, tgsz * ocs], CONV_DT, tag, bufs=bufs)
                dma(w[:], w_dram[kt * 128:(kt + 1) * 128,
                                 tg * tgsz * ocs:(tg + 1) * tgsz * ocs])
                return w[:]
            return src

        # conv1: 512 -> 256, output into padded tiles for conv2
        pad1_t = []
        for mt in range(2):
            t = T(pp, [128, SPP], CONV_DT, f"pad1_{mt}")
            nc.vector.memset(t[:], 0.0)
            pad1_t.append(t)

        def c1_out(mt, ps):
            inner = pad1_t[mt][:].rearrange("p (a b) -> p a b", a=PW, b=PW)[
                :, 2:2 + R1, 2:2 + S]
            nc.scalar.activation(inner, ps[:].rearrange("p (a b) -> p a b", a=R1, b=S),
                                 ACT.Relu, bias=bsb[:, mt:mt + 1])

        conv(xpad_t, stream_w(w1t, IC), 4, 2, IC, R1, c1_out)

        pad2_t = []
        for mt in range(2):
            t = T(pp, [128, SPP], CONV_DT, f"pad2_{mt}")
            nc.vector.memset(t[:], 0.0)
            pad2_t.append(t)

        def c2_out(mt, ps):
            inner = pad2_t[mt][:].rearrange("p (a b) -> p a b", a=PW, b=PW)[
                :, 2:2 + R2, 2:2 + S]
            nc.scalar.activation(inner, ps[:].rearrange("p (a b) -> p a b", a=R2, b=S),
                                 ACT.Relu, bias=bsb[:, 2 + mt:3 + mt])

        conv(pad1_t, stream_w(w2t, IC, bufs=5, tag="w2_stream"),
             2, 2, IC, R2, c2_out)

        x3_t = []
        for mt in range(4):
            t = T(pp, [128, C3], PAIR_DT, f"x3_{mt}")
            x3_t.append(t)

        def c3_out(mt, ps):
            nc.scalar.activation(x3_t[mt][:], ps[:], ACT.Relu,
                                 bias=bsb[:, 4 + mt:5 + mt])

        conv(pad2_t, stream_w(w3t, EMB), 2, 4, EMB, R3, c3_out)

        # ---- S7: remaining pair features + classifier ----
        # x3T [242, 512]
        x3T_t = []
        for i, (off, sz) in enumerate(_ts(SP3_TILES)):
            t = T(pp, [sz, EMB], PAIR_DT, f"x3T{i}")
            x3T_t.append(t)
        for i, (off, sz) in enumerate(_ts(SP3_TILES)):
            for src in range(4):
                ps = T(pps, [sz, 64], FP32, "ps")
                psb = ps[:].bitcast(PAIR_DT)
                nc.tensor.transpose(psb, x3_t[src][:, off:off + sz],
                                    identp_t[:, :])
                nc.vector.tensor_copy(x3T_t[i][:, src * 128:(src + 1) * 128], psb)

        for mt in range(4):
            ps = T(pps, [128, CAP], FP32, "ps")
            for i, (off, sz) in enumerate(_ts(SP3_TILES)):
                nc.tensor.matmul(ps[:], x3T_t[i][:, mt * 128:(mt + 1) * 128],
                                 sm_t[i][:], start=(i == 0), stop=(i == 1))
            t = T(pp, [128, CAP], PAIR_DT, f"featT{8 + mt}")
            nc.vector.tensor_copy(t[:], ps[:])
            featT[8 + mt] = t

        ps_ht = [T(pps, [128, CAP], FP32, "ps") for _ in range(8)]
        for kt in range(16):
            w = T(pst, [128, 2 * EMB], PAIR_DT, "wht_stream", bufs=6)
            dma(w[:], wht[kt * 128:(kt + 1) * 128, :])
            for mt in range(8):
                nc.tensor.matmul(ps_ht[mt][:], w[:, mt * 128:(mt + 1) * 128],
                                 featT[kt][:], start=(kt == 0), stop=(kt == 15))
        htT_t = []
        for mt in range(8):
            t = T(pp, [128, CAP], PAIR_DT, f"htT{mt}")
            nc.scalar.activation(t[:], ps_ht[mt][:], ACT.Tanh,
                                 bias=bsb[:, 8 + mt:9 + mt])
            htT_t.append(t)

        ps_out = T(pps, [97, CAP], FP32, "ps")
        for kt in range(8):
            nc.tensor.matmul(ps_out[:], wbil_t[:, kt * 97:(kt + 1) * 97],
                             htT_t[kt][:], start=(kt == 0), stop=(kt == 7))
        out_t = T(pp, [97, CAP], FP32, "out")
        nc.vector.tensor_scalar_add(out_t[:], ps_out[:], bsb[0:97, 16:17])
        # split the output across both queues so the drain halves
        dma(outt[0:49, :], out_t[0:49, :])
        dmas(outt[49:97, :], out_t[49:97, :])

    nc.compile()
    return nc


_PROG = None


def _get_prog():
    global _PROG
    if _PROG is None:
        _PROG = build_program()
    return _PROG


def _np(dt):
    return _NPDT[dt]


def _drpack(a, np_dt):
    """[K, F] -> DoubleRow rows (kpair, p), cols (i, F); K % 256 == 0."""
    K, F = a.shape
    return np.ascontiguousarray(
        a.reshape(K // 256, 2, 128, F).transpose(0, 2, 1, 3)
        .reshape(K // 2, 2 * F)).astype(np_dt)


def _kpcat(a):
    """[n*128, F] -> [128, n*F]: slab-per-128-rows packed side by side."""
    n = a.shape[0] // 128
    return np.ascontiguousarray(
        np.concatenate([a[i * 128:(i + 1) * 128] for i in range(n)], axis=1))


def _shared_inputs(inputs):
    f32 = np.float32
    sh = {}
    sh["wtrans"] = _drpack(np.asarray(inputs["W_trans"], f32) * WS, _np(FP8))
    sh["btrans"] = np.ascontiguousarray(
        inputs["b_trans"], f32).reshape(1, EMB) * WS
    sh["g3"] = np.kron(np.eye(E, dtype=f32),
                       np.ones((M, 1), f32)).astype(_np(GRAPH_DT))
    sh["_gspan"] = np.kron(np.eye(L, dtype=f32),
                           np.ones((LS, 1), f32)).astype(_np(BF16))
    sh["ident"] = np.eye(128, dtype=f32)
    sh["identp"] = np.eye(128, dtype=_np(PAIR_DT))
    wrel_all = np.concatenate(
        [np.asarray(inputs["W_rel"], f32).transpose(1, 0, 2).reshape(NF, 4 * EMB),
         np.asarray(inputs["W_self"], f32)], axis=1)   # [532, 5*512]
    sh["wrel"] = np.ascontiguousarray(wrel_all).astype(_np(GRAPH_DT))
    sh["brgcn"] = np.ascontiguousarray(inputs["b_rgcn"], f32).reshape(1, EMB)

    def packw(w, ict, oc):
        # [oc, ic, 5, 5] -> rows ic, cols (tap, oc)
        return np.ascontiguousarray(
            np.asarray(w, f32).transpose(1, 2, 3, 0).reshape(ict, 25 * oc),
            _np(CONV_DT))

    w1 = np.asarray(inputs["conv1_w"], f32)
    w2 = np.asarray(inputs["conv2_w"], f32)
    w3 = np.asarray(inputs["conv3_w"], f32)
    sh["w1t"] = packw(w1, EMB, IC)
    sh["_w1t_f"] = packw(w1[:, :, ::-1, ::-1], EMB, IC)
    sh["w2t"] = packw(w2, IC, IC)
    sh["_w2t_f"] = packw(w2[:, :, ::-1, ::-1], IC, IC)
    sh["w3t"] = packw(w3, IC, EMB)
    sh["_w3t_f"] = packw(w3[:, :, ::-1, ::-1], IC, EMB)
    # biasT rows: b1(2) b2(2) b3(4) bht(8) bbil(1)
    bt = np.zeros((NBIAS, 128), f32)
    bt[0:2] = np.asarray(inputs["conv1_b"], f32).reshape(2, 128)
    bt[2:4] = np.asarray(inputs["conv2_b"], f32).reshape(2, 128)
    bt[4:8] = np.asarray(inputs["conv3_b"], f32).reshape(4, 128)
    bt[8:16] = np.asarray(inputs["ht_b"], f32).reshape(8, 128)
    bt[16, 0:97] = np.asarray(inputs["bil_b"], f32).reshape(97)
    sh["biasT"] = bt
    sh["wht"] = np.ascontiguousarray(inputs["ht_W"], _np(PAIR_DT))
    sh["wbil"] = np.ascontiguousarray(
        np.asarray(inputs["bil_W"], f32).reshape(8, 128, 97)
        .transpose(1, 0, 2).reshape(128, 8 * 97)).astype(_np(PAIR_DT))
    return sh


def _routing(hts_b, hh):
    """Pair columns routed to half hh: by head-entity row of the 22x22 map."""
    hi = hts_b[:, 0]
    return np.nonzero(hi <= 10 if hh == 0 else hi >= 11)[0]


def _core_inputs(inputs, shared, b, hh):
    f32 = np.float32
    X = np.asarray(inputs["sequence_output"][b], f32)
    att = np.asarray(inputs["attention"][b], f32)
    adj = np.asarray(inputs["adjacency"][b], f32)
    mi = np.asarray(inputs["mention_idx"][b]).astype(np.int64)
    ls = np.asarray(inputs["link_start"][b]).reshape(-1).astype(np.int64)
    ntypes = np.asarray(inputs["node_types"][b]).astype(np.int64)
    hts = np.asarray(inputs["hts"][b]).astype(np.int64)

    m = {k: v for k, v in shared.items() if not k.startswith("_")}
    if hh == 1:
        # 180-degree flip: relabel entities e -> 21-e (and their mention
        # nodes) so this core's rows 0..10 are the true rows 21..11.
        eperm = np.arange(E)[::-1]
        mi = mi[eperm]
        nperm = np.concatenate([
            eperm,
            E + np.repeat(eperm, M) * M + np.tile(np.arange(M), E),
            np.arange(E + E * M, NN)])
        adj = adj[:, nperm][:, :, nperm]
        ntypes = ntypes[nperm]
        m["w1t"] = shared["_w1t_f"]
        m["w2t"] = shared["_w2t_f"]
        m["w3t"] = shared["_w3t_f"]
    mf = mi.reshape(-1)
    xt_full = np.zeros((H, C + 128), f32)
    xt_full[:, :C] = X.T
    xt_full[:, C:C + E * M] = X[mf].T
    m["xt"] = _drpack(xt_full, _np(FP8))
    pos = ls[:, None] + np.arange(LS)
    # xrow pack: [xspan | attl | gspan], 4 span tiles side by side
    xrow = np.zeros((SPAN_ROWS, XR), _np(BF16))
    xrow[:, 0:H] = np.ascontiguousarray(X[pos.reshape(-1)]).astype(_np(BF16))
    attl = np.empty((SPAN_ROWS, NH * LS), f32)
    for l in range(L):
        blk = att[:, pos[l], :][:, :, pos[l]]           # [12, 16i, 16j]
        attl[l * LS:(l + 1) * LS, :] = blk.transpose(2, 0, 1).reshape(LS, NH * LS)
    xrow[:, H:H + NH * LS] = attl.astype(_np(BF16))
    xrow[:, H + NH * LS:XR] = shared["_gspan"]
    m["xrow"] = xrow
    # attm pack: mention att rows (792 padded to 1024) + gmat selector cols
    rows = att[:, mf, :].transpose(1, 0, 2).reshape(ATTM_ROWS, C)
    am = np.zeros((1024, CA), f32)
    am[:ATTM_ROWS, :C] = rows
    am[:ATTM_ROWS, C:C + E] = np.kron(np.eye(E, dtype=f32),
                                      np.ones((M * NH, 1), f32))
    m["attm"] = _drpack(am, _np(FP8))
    # slim adjacency: [j, (r, i<22)] for 4 relations + self-loop identity
    adjs = np.empty((NN, AC), f32)
    for r in range(4):
        adjs[:, r * E:(r + 1) * E] = adj[r].T[:, 0:E]
    adjs[:, 4 * E:AC] = np.eye(NN, E, dtype=f32)
    m["adjt"] = np.ascontiguousarray(adjs)
    m["typ"] = np.ascontiguousarray(
        np.asarray(inputs["type_embed"], f32)[ntypes]).astype(_np(GRAPH_DT))
    idx = _routing(hts, hh)
    pr = hts[idx]
    if hh == 1:
        pr = (E - 1) - pr
    n = len(idx)
    shm = np.zeros((E, CAP), f32)
    shm[pr[:, 0], np.arange(n)] = 1.0
    stm = np.zeros((E, CAP), f32)
    stm[pr[:, 1], np.arange(n)] = 1.0
    smm = np.zeros((C3, CAP), f32)
    smm[pr[:, 0] * S + pr[:, 1], np.arange(n)] = 1.0
    m["sh"] = np.ascontiguousarray(shm, _np(PAIR_DT))
    m["st"] = np.ascontiguousarray(stm, _np(PAIR_DT))
    m["sm"] = np.ascontiguousarray(smm, _np(PAIR_DT))
    return m


def kernel(**inputs):
    nc = _get_prog()
    shared = _shared_inputs(inputs)
    in_maps = []
    for b in range(B):
        for hh in range(2):
            in_maps.append(_core_inputs(inputs, shared, b, hh))
    res = run_bass_kernel_spmd(nc, in_maps, core_ids=list(range(8)))
    out = np.empty((B, P, 97), np.float32)
    for b in range(B):
        hts_b = np.asarray(inputs["hts"][b]).astype(np.int64)
        for hh in range(2):
            idx = _routing(hts_b, hh)
            r = np.asarray(res.results[2 * b + hh]["outt"], np.float32)
            out[b, idx, :] = r[:, :len(idx)].T
    return out
